# revision 1
# baseline (speedup 1.0000x reference)
"""Trainium2 Bass kernel for equivariant multihead attention.

Math (per batch b, query point i, coset s1, channel c):
    logit[j,s2] = sum_g pairwise_g[b,i,j,s1,s2,g]*w_g[c,g]
                  + w_y[c,0]*y[b,j,s2,c] + w_y[c,1]*y[b,i,s1,c] + b_g[c] + b_y[c]
    att = exp(logit)*mask[b,j,s2];  att /= sum_{j,s2} att
    out = (y[b,i,s1,c] + sum_{j,s2} att*y[b,j,s2,c]) * mask[b,i,s1]  @ w_lin.T

The query-side term and the biases are constant over the key dims (j,s2), so
they cancel in the normalization and are dropped.  The key-side factor
exp(w_y[c,0]*y[b,j,s2,c])*mask[b,j,s2] is a tiny per-batch table KD (and
KD*y = KN), precomputed on host.  Per (b,i) block the device computes
    E[(s1,s2,c), j] = exp(sum_g G_T[(s1,s2,g), j] * w_g[c,g])
    den_part[(s1,s2,c)] = sum_j E * KD_T     (fused multiply-reduce)
    num_part[(s1,s2,c)] = sum_j E * KN_T
and one final PE matmul sums the partials over s2.  Host finishes with the
residual add, query mask, and the c_in->c_out linear (all tiny).

Sharding: query dim i is split 8 ways (16 i x 4 b = 64 blocks per core).

Implementation notes (walrus on this stack allows only ONE sync wait per
Matmult / DMA / STT instruction, and ~12 on the final drain):
  * ALL inputs ship as ONE dram "blob" per core, loaded by 7 big
    column-range DMAs into a single SBUF plane -> every DMA is the first
    on its HW-DGE queue (no proc-predecessor wait) and descriptor runs are
    ~17KB contiguous (max DMA efficiency).  The final store is the 8th DMA
    (queue 7, also virgin).
  * tiny "spacer" ops make each engine observe cross-engine ticks ahead of
    the real instructions, so those carry at most one wait each.
"""

import numpy as np

import concourse.bacc as bacc
import concourse.tile as tile
from concourse import mybir
from concourse.bass_utils import run_bass_kernel_spmd

B, N, S, CIN, COUT, GDIM = 4, 128, 8, 8, 8, 7
NCORES = 8
ISHARD = N // NCORES          # 16 query points per core
NBLK = B * ISHARD             # 64 (b,i) blocks per core
PW = S * GDIM                 # 56: free width of one s1 slice
QW = 2 * PW                   # 112: free width of one transpose quarter
NQ = 4                        # quarters per block
BW = NQ * QW                  # 448 floats per (j, block)
NCOL = NBLK * NQ              # 256 partial columns per half

# blob column layout: [ident | kd | kn | bd | sind | pg blocks]
IDENT0 = 0
KD0 = 128
KN0 = KD0 + B * N             # 640
BD0 = KN0 + B * N             # 1152
SIND0 = BD0 + 128             # 1280
CONSTW = SIND0 + 16           # 1296
TOTW = CONSTW + NBLK * BW     # 29968

# blocks covered by each of the 7 input DMAs (first also carries consts;
# earlier ones smaller for a faster pipeline ramp)
SUPER_BLOCKS = (4, 6, 8, 10, 11, 12, 13)

F32 = mybir.dt.float32

# per-quarter engine assignment (balance tuning): PSUM->SBUF copy of the
# transposed quarter, and den/num fused multiply-reduces.  bacc's
# split_sync_waits legalizes any multi-wait instructions this creates.
# (gpsimd cannot run scalar_tensor_tensor: not a valid Pool-engine opcode)
COPY_ENG = ("act", "act", "act", "dve")
DEN_ENG = ("dve", "dve", "dve", "dve")
NUM_ENG = ("dve", "dve", "dve", "dve")

_PROGRAM_CACHE = {}


def _build_program(nblk=NBLK, loop_reps=1):
    """loop_reps>1 wraps the main loop in a hardware For_i that re-runs the
    full pass (including the input DMAs) on the same data -- used only for
    timing: wall(loop_reps=R) - wall(loop_reps=1) isolates device time from
    the ~100ms axon dispatch/transfer overhead."""
    nc = bacc.Bacc("TRN2", target_bir_lowering=False, debug=False,
                   num_devices=NCORES)

    blob_d = nc.dram_tensor("blob", (N, TOTW), F32, kind="ExternalInput").ap()
    out_s = nc.dram_tensor("out_s", (16, 2 * NCOL), F32,
                           kind="ExternalOutput").ap()

    # per-super [start_block, end_block) and column ranges
    supers = []
    blk0 = 0
    for nb in SUPER_BLOCKS:
        if blk0 >= nblk:
            break
        nb = min(nb, nblk - blk0)
        c0 = 0 if blk0 == 0 else CONSTW + blk0 * BW
        c1 = CONSTW + (blk0 + nb) * BW
        supers.append((blk0, blk0 + nb, c0, c1))
        blk0 += nb

    with tile.TileContext(nc) as tc:
        with (
            tc.tile_pool(name="consts", bufs=1) as consts,
            tc.tile_pool(name="gtpool", bufs=4) as gtpool,
            tc.tile_pool(name="epool", bufs=4) as epool,
            tc.tile_pool(name="psA", bufs=4, space="PSUM") as psA,
            tc.tile_pool(name="psB", bufs=2, space="PSUM") as psB,
            tc.tile_pool(name="psC", bufs=1, space="PSUM") as psC,
        ):
            g_all = consts.tile([N, TOTW], F32)
            ident = g_all[:, IDENT0:IDENT0 + 128]
            bd = g_all[0:QW, BD0:BD0 + 128]
            sind = g_all[:, SIND0:SIND0 + 16]

            buf_dve = consts.tile([128, 2 * NCOL], F32)
            nc.vector.memset(buf_dve, 0.0)

            NDUM = 8
            dummies = [consts.tile([128, 1], F32, name=f"dum{i}")
                       for i in range(NDUM)]
            dum_idx = [0]
            s_sb = consts.tile([16, 2 * NCOL], F32)

            def stt_reduce(eng, e_q, table, col_ap):
                dum = dummies[dum_idx[0] % NDUM]
                dum_idx[0] += 1
                engine = nc.vector if eng == "dve" else nc.gpsimd
                engine.scalar_tensor_tensor(
                    dum.broadcast_to(e_q.shape), e_q, 0.0, table,
                    op0=mybir.AluOpType.bypass, op1=mybir.AluOpType.mult,
                    accum_out=col_ap)

            def main_pass():
              for (b0, b1, c0, c1) in supers:
                nc.sync.dma_start(g_all[:, c0:c1], blob_d[:, c0:c1])
              for (b0, b1, c0, c1) in supers:
                for blk in range(b0, b1):
                    b = blk // ISHARD
                    gcol = CONSTW + blk * BW
                    kd_b = g_all[:, KD0 + b * N:KD0 + (b + 1) * N]
                    kn_b = g_all[:, KN0 + b * N:KN0 + (b + 1) * N]

                    gt_cat = gtpool.tile([QW, NQ, 128], F32, tag="gt")
                    for q in range(NQ):
                        gt_ps = psA.tile([QW, 128], F32, tag="gtps")
                        nc.tensor.transpose(
                            gt_ps,
                            g_all[:, gcol + QW * q:gcol + QW * (q + 1)],
                            ident)
                        if COPY_ENG[q] == "act":
                            nc.scalar.copy(gt_cat[:, q, :], gt_ps)
                        else:
                            nc.vector.tensor_copy(gt_cat[:, q, :], gt_ps)

                    l_ps = psB.tile([128, NQ, 128], F32, tag="lps")
                    nc.tensor.matmul(l_ps, lhsT=bd, rhs=gt_cat,
                                     start=True, stop=True)

                    e_t = epool.tile([128, NQ, 128], F32, tag="e")
                    nc.scalar.activation(e_t, l_ps,
                                         mybir.ActivationFunctionType.Exp)

                    for q in range(NQ):
                        col = blk * NQ + q
                        e_q = e_t[:, q, :]
                        stt_reduce(DEN_ENG[q], e_q, kd_b,
                                   buf_dve[:, col:col + 1])
                        stt_reduce(NUM_ENG[q], e_q, kn_b,
                                   buf_dve[:, NCOL + col:NCOL + col + 1])

            if loop_reps > 1:
                with tc.For_i(0, loop_reps, 1,
                              hint_engines=(mybir.EngineType.PE,
                                            mybir.EngineType.Activation,
                                            mybir.EngineType.DVE,
                                            mybir.EngineType.SP)):
                    main_pass()
            else:
                main_pass()

            # sum the (h,s2,c) j-partials over s2 -> (h,c)
            s_ps = psC.tile([16, 2 * NCOL], F32)
            nc.tensor.matmul(s_ps, lhsT=sind, rhs=buf_dve,
                             start=True, stop=True)
            nc.scalar.copy(s_sb, s_ps)
            nc.sync.dma_start(out_s, s_sb)   # 8th DMA -> virgin queue 7

    nc.compile()   # bacc: register alloc + split_sync_waits (1-wait limit)
    return nc


def _get_program(nblk=NBLK, loop_reps=1):
    key = ("nc", nblk, loop_reps)
    if key not in _PROGRAM_CACHE:
        _PROGRAM_CACHE[key] = _build_program(nblk, loop_reps)
    return _PROGRAM_CACHE[key]


def _host_prep(pairwise_g, coset_functions, mask, w_y, w_g):
    """Build the per-core input blobs."""
    y = coset_functions.astype(np.float32)          # (B, N, S, C) keys
    maskf = mask.astype(np.float32)
    ey = np.exp(y * w_y[:, 0]) * maskf[..., None]   # (B, j, s2, c)
    kn = ey * y
    # rows (h, s2, c) with h in {0,1} duplicated; cols j
    kd_t = np.tile(ey.transpose(0, 2, 3, 1).reshape(B, S * CIN, N), (1, 2, 1))
    kn_t = np.tile(kn.transpose(0, 2, 3, 1).reshape(B, S * CIN, N), (1, 2, 1))

    bd = np.zeros((128, 128), np.float32)
    for pl in range(16):
        for g in range(GDIM):
            for c in range(CIN):
                bd[pl * GDIM + g, pl * CIN + c] = w_g[c, g]

    sind = np.zeros((128, 16), np.float32)
    for h in range(2):
        for s2 in range(S):
            for c in range(CIN):
                sind[h * 64 + s2 * CIN + c, h * CIN + c] = 1.0

    consts_plane = np.empty((N, CONSTW), np.float32)
    consts_plane[:, IDENT0:IDENT0 + 128] = np.eye(128, dtype=np.float32)
    consts_plane[:, KD0:KD0 + B * N] = kd_t.transpose(1, 0, 2).reshape(128, -1)
    consts_plane[:, KN0:KN0 + B * N] = kn_t.transpose(1, 0, 2).reshape(128, -1)
    consts_plane[:, BD0:BD0 + 128] = bd
    consts_plane[:, SIND0:SIND0 + 16] = sind

    in_maps = []
    for k in range(NCORES):
        sl = slice(ISHARD * k, ISHARD * (k + 1))
        pg_core = pairwise_g[:, sl].reshape(NBLK, N, BW)
        blob = np.empty((N, TOTW), np.float32)
        blob[:, :CONSTW] = consts_plane
        blob[:, CONSTW:] = pg_core.transpose(1, 0, 2).reshape(N, NBLK * BW)
        in_maps.append({"blob": blob})
    return in_maps


def _host_finish(s_list, coset_functions, mask, w_lin):
    """Decode per-core (16, 512) outputs into the full result."""
    y = np.asarray(coset_functions, dtype=np.float32)
    maskf = np.asarray(mask).astype(np.float32)
    out = np.empty((B, N, S, COUT), np.float32)
    for k in range(NCORES):
        s = s_list[k]
        den = s[:, :NCOL].reshape(2, CIN, NBLK, NQ)
        num = s[:, NCOL:].reshape(2, CIN, NBLK, NQ)
        # (h, c, blk, q) -> (blk, s1 = 2q + h, c)
        den = den.transpose(2, 3, 0, 1).reshape(NBLK, S, CIN)
        num = num.transpose(2, 3, 0, 1).reshape(NBLK, S, CIN)
        sl = slice(ISHARD * k, ISHARD * (k + 1))
        y_q = y[:, sl].reshape(NBLK, S, CIN)
        m_q = maskf[:, sl].reshape(NBLK, S)
        res = (y_q + num / den) * m_q[..., None]
        res = res @ w_lin.T
        out[:, sl] = res.reshape(B, ISHARD, S, COUT)
    return out


def kernel(pairwise_g, coset_functions, mask, w_y, b_y, w_g, b_g, w_lin):
    pairwise_g = np.asarray(pairwise_g, dtype=np.float32)
    coset_functions = np.asarray(coset_functions, dtype=np.float32)
    mask = np.asarray(mask)
    w_y = np.asarray(w_y, dtype=np.float32)
    w_g = np.asarray(w_g, dtype=np.float32)
    w_lin = np.asarray(w_lin, dtype=np.float32)

    nc = _get_program()
    in_maps = _host_prep(pairwise_g, coset_functions, mask, w_y, w_g)
    res = run_bass_kernel_spmd(nc, in_maps, core_ids=list(range(NCORES)))
    s_list = [r["out_s"] for r in res.results]
    return _host_finish(s_list, coset_functions, mask, w_lin)



# revision 2
# speedup vs baseline: 6.2508x; 6.2508x over previous
"""Trainium2 Bass kernel for equivariant multihead attention.

Math (per batch b, query point i, coset s1, channel c):
    logit[j,s2] = sum_g pairwise_g[b,i,j,s1,s2,g]*w_g[c,g]
                  + w_y[c,0]*y[b,j,s2,c] + w_y[c,1]*y[b,i,s1,c] + b_g[c] + b_y[c]
    att = exp(logit)*mask[b,j,s2];  att /= sum_{j,s2} att
    out = (y[b,i,s1,c] + sum_{j,s2} att*y[b,j,s2,c]) * mask[b,i,s1]  @ w_lin.T

The query-side term and the biases are constant over the key dims (j,s2), so
they cancel in the normalization and are dropped.  The key-side factor
exp(w_y[c,0]*y[b,j,s2,c])*mask[b,j,s2] is FOLDED INTO THE LOGITS via a second
PE matmul accumulating into the same PSUM bank: 8 "y-feature" rows per s2
(feature k carries w_y[k,0]*y[b,j,s2,k] + logmask[b,j,s2], weight delta(k,c))
add exactly w_y[c,0]*y[b,j,s2,c] + logmask to every (h,s2,c) logit row.
So E' = exp(L') already includes the mask and key factor, and per (b,i) block
    den[(h,s2,c), q] = sum_j E'          (plain row-sum: DVE segmented reduce
                                          or act-accum during exp)
    num[(h,s2,c), q] = sum_j E' * Y      (Y[(h,s2,c), j] = y[b,j,s2,c];
                                          one STT per quarter on DVE)
and one final PE matmul sums the partials over s2.  Host finishes with the
residual add, query mask, and the c_in->c_out linear (all tiny).

pairwise_g is pre-transposed ON HOST to [(h,s2,g)=112 rows, (blk,q,j)] bf16 so
no PE transposes / PSUM->SBUF copies are needed on device; bf16 also halves
the HBM traffic and runs the PE at 1 cycle/row.

Sharding: query dim i is split 8 ways (16 i x 4 b = 64 blocks per core).
"""

import numpy as np
import ml_dtypes

import concourse.bacc as bacc
import concourse.tile as tile
from concourse import mybir
from concourse.bass_utils import run_bass_kernel_spmd

B, N, S, CIN, COUT, GDIM = 4, 128, 8, 8, 8, 7
NCORES = 8
ISHARD = N // NCORES          # 16 query points per core
NBLK = B * ISHARD             # 64 (b,i) blocks per core
NQ = 4                        # s1 pairs per block
BW = NQ * 128                 # 512 pg columns per block
NCOL = NBLK * NQ              # 256 partial columns per half
PGROWS = 2 * S * GDIM         # 112 rows: (h, s2, g)
EXTROWS = S * CIN             # 64 rows: (s2, k) y-features (mask folded in)
LOGMASK0 = -50.0              # logit offset for masked keys

# blocks covered by each of the 7 pg DMAs (earlier ones smaller for ramp)
SUPER_BLOCKS = (4, 6, 8, 10, 11, 12, 13)

# den computed by DVE segmented reduce ("dve") or act exp+accum ("act"),
# chosen per block to balance the two engines.
DEN_ACT_FRAC = 0.5

F32 = mybir.dt.float32
BF16 = mybir.dt.bfloat16

# consts_bf16 column layout
BD0 = 0                        # bd [112, 128]
BDE0 = 128                     # bdext [64, 128]
Y0 = 256                       # Y tables [128, 128] x B
EXT0 = Y0 + B * 128            # extrep [64, 512] x B
CBW = EXT0 + B * BW            # total bf16 consts cols
# consts_f32: sind [128, 16]
CFW = 16

_PROGRAM_CACHE = {}


def _den_engine(blk):
    # spread act-den blocks evenly through the pass
    return "act" if (blk * DEN_ACT_FRAC) % 1.0 + DEN_ACT_FRAC > 1.0 - 1e-9 else "dve"


def _build_program(nblk=NBLK, loop_reps=1):
    """loop_reps>1 wraps the main loop in a hardware For_i that re-runs the
    full pass (including the input DMAs) on the same data -- used only for
    timing: wall(loop_reps=R) - wall(loop_reps=1) isolates device time from
    the ~100ms axon dispatch/transfer overhead."""
    nc = bacc.Bacc("TRN2", target_bir_lowering=False, debug=False,
                   num_devices=NCORES)

    pg_d = nc.dram_tensor("pg", (PGROWS, NBLK * BW), BF16,
                          kind="ExternalInput").ap()
    cb_d = nc.dram_tensor("cb", (128, CBW), BF16, kind="ExternalInput").ap()
    cf_d = nc.dram_tensor("cf", (128, CFW), F32, kind="ExternalInput").ap()
    out_s = nc.dram_tensor("out_s", (16, 2 * NCOL), F32,
                           kind="ExternalOutput").ap()

    # per-super [start_block, end_block)
    supers = []
    blk0 = 0
    for nb in SUPER_BLOCKS:
        if blk0 >= nblk:
            break
        nb = min(nb, nblk - blk0)
        supers.append((blk0, blk0 + nb))
        blk0 += nb

    with tile.TileContext(nc) as tc:
        with (
            tc.tile_pool(name="consts", bufs=1) as consts,
            tc.tile_pool(name="epool", bufs=4) as epool,
            tc.tile_pool(name="psA", bufs=4, space="PSUM") as psA,
            tc.tile_pool(name="psC", bufs=1, space="PSUM") as psC,
        ):
            pg_all = consts.tile([PGROWS, nblk * BW], BF16)
            cb = consts.tile([128, CBW], BF16)
            cf = consts.tile([128, CFW], F32)

            bd = cb[0:PGROWS, BD0:BD0 + 128]
            bdext = cb[0:EXTROWS, BDE0:BDE0 + 128]
            sind = cf[:, 0:16]

            buf = consts.tile([128, 2 * NCOL], F32)

            NDUM = 8
            dummies = [consts.tile([128, 1], BF16, name=f"dum{i}")
                       for i in range(NDUM)]
            dum_idx = [0]
            s_sb = consts.tile([16, 2 * NCOL], F32)

            def main_pass():
                nc.sync.dma_start(cb, cb_d)
                nc.sync.dma_start(cf, cf_d)
                for (b0, b1) in supers:
                    nc.sync.dma_start(pg_all[:, b0 * BW:b1 * BW],
                                      pg_d[:, b0 * BW:b1 * BW])
                for (b0, b1) in supers:
                    for blk in range(b0, b1):
                        b = blk // ISHARD
                        y_b = cb[:, Y0 + b * 128:Y0 + (b + 1) * 128]
                        ext_b = cb[0:EXTROWS, EXT0 + b * BW:EXT0 + (b + 1) * BW]
                        pg_blk = pg_all[:, blk * BW:(blk + 1) * BW]

                        l_ps = psA.tile([128, NQ, 128], F32, tag="lps")
                        nc.tensor.matmul(l_ps, lhsT=bd, rhs=pg_blk,
                                         start=True, stop=False)
                        nc.tensor.matmul(l_ps, lhsT=bdext, rhs=ext_b,
                                         start=False, stop=True)

                        e_t = epool.tile([128, NQ, 128], BF16, tag="e")
                        dmode = _den_engine(blk)
                        if dmode == "act":
                            for q in range(NQ):
                                nc.scalar.activation(
                                    e_t[:, q, :], l_ps[:, q, :],
                                    mybir.ActivationFunctionType.Exp,
                                    accum_out=buf[:, blk * NQ + q:
                                                  blk * NQ + q + 1])
                        else:
                            nc.scalar.activation(
                                e_t, l_ps, mybir.ActivationFunctionType.Exp)
                            nc.vector.tensor_reduce(
                                buf[:, blk * NQ:(blk + 1) * NQ], e_t,
                                axis=mybir.AxisListType.X,
                                op=mybir.AluOpType.add)
                        for q in range(NQ):
                            col = NCOL + blk * NQ + q
                            dum = dummies[dum_idx[0] % NDUM]
                            dum_idx[0] += 1
                            nc.vector.scalar_tensor_tensor(
                                dum.broadcast_to((128, 128)), e_t[:, q, :],
                                0.0, y_b,
                                op0=mybir.AluOpType.bypass,
                                op1=mybir.AluOpType.mult,
                                accum_out=buf[:, col:col + 1])

            if loop_reps > 1:
                with tc.For_i(0, loop_reps, 1,
                              hint_engines=(mybir.EngineType.PE,
                                            mybir.EngineType.Activation,
                                            mybir.EngineType.DVE,
                                            mybir.EngineType.SP)):
                    main_pass()
            else:
                main_pass()

            # sum the (h,s2,c) j-partials over s2 -> (h,c)
            s_ps = psC.tile([16, 2 * NCOL], F32)
            nc.tensor.matmul(s_ps, lhsT=sind, rhs=buf, start=True, stop=True)
            nc.scalar.copy(s_sb, s_ps)
            nc.sync.dma_start(out_s, s_sb)

    nc.compile()
    return nc


def _get_program(nblk=NBLK, loop_reps=1):
    key = ("nc", nblk, loop_reps)
    if key not in _PROGRAM_CACHE:
        _PROGRAM_CACHE[key] = _build_program(nblk, loop_reps)
    return _PROGRAM_CACHE[key]


def _host_prep(pairwise_g, coset_functions, mask, w_y, w_g):
    """Build the per-core input arrays."""
    y = coset_functions.astype(np.float32)          # (B, N, S, C) keys
    maskf = mask.astype(np.float32)
    logmask = np.where(mask, 0.0, LOGMASK0).astype(np.float32)  # (B, j, s2)

    # bd [112, 128]: (h,s2,g) -> (h,s2,c) per-plane w_g
    bd = np.zeros((PGROWS, 128), np.float32)
    for pl in range(16):
        for g in range(GDIM):
            for c in range(CIN):
                bd[pl * GDIM + g, pl * CIN + c] = w_g[c, g]

    # bdext [64, 128]: y-feature row (s2,k) -> (h,s2,c) with weight d(k,c)
    bdext = np.zeros((EXTROWS, 128), np.float32)
    for h in range(2):
        for s2 in range(S):
            for c in range(CIN):
                bdext[s2 * CIN + c, (h * S + s2) * CIN + c] = 1.0

    # ext[b][(s2,k), j] = w_y[k,0]*y[b,j,s2,k] + logmask[b,j,s2]
    ext = (y.transpose(0, 2, 3, 1) * w_y[:, 0][None, None, :, None]
           + logmask.transpose(0, 2, 1)[:, :, None, :])  # (B, s2, k, j)
    ext = ext.reshape(B, EXTROWS, N)

    # Y[b][(h,s2,c), j] = y[b,j,s2,c]
    ytab = np.tile(y.transpose(0, 2, 3, 1).reshape(B, S * CIN, N), (1, 2, 1))

    sind = np.zeros((128, 16), np.float32)
    for h in range(2):
        for s2 in range(S):
            for c in range(CIN):
                sind[h * 64 + s2 * CIN + c, h * CIN + c] = 1.0

    cb = np.zeros((128, CBW), np.float32)
    cb[0:PGROWS, BD0:BD0 + 128] = bd
    cb[0:EXTROWS, BDE0:BDE0 + 128] = bdext
    for b in range(B):
        cb[:, Y0 + b * 128:Y0 + (b + 1) * 128] = ytab[b]
        cb[0:EXTROWS, EXT0 + b * BW:EXT0 + (b + 1) * BW] = np.tile(
            ext[b], (1, NQ)).reshape(EXTROWS, NQ, N).transpose(0, 1, 2).reshape(
            EXTROWS, BW)
    cb = cb.astype(ml_dtypes.bfloat16)

    cf = np.zeros((128, CFW), np.float32)
    cf[:, 0:16] = sind

    in_maps = []
    for k in range(NCORES):
        sl = slice(ISHARD * k, ISHARD * (k + 1))
        pgc = pairwise_g[:, sl]                      # (B, 16, j, s1, s2, g)
        pgc = pgc.reshape(B, ISHARD, N, NQ, 2, S, GDIM)
        # -> (b, i, h, s2, g, q, j)
        pgc = pgc.transpose(0, 1, 4, 5, 6, 3, 2)
        pgc = pgc.reshape(NBLK, PGROWS, NQ * N)
        pg = np.ascontiguousarray(pgc.transpose(1, 0, 2)).reshape(
            PGROWS, NBLK * BW).astype(ml_dtypes.bfloat16)
        in_maps.append({"pg": pg, "cb": cb, "cf": cf})
    return in_maps


def _host_finish(s_list, coset_functions, mask, w_lin):
    """Decode per-core (16, 512) outputs into the full result."""
    y = np.asarray(coset_functions, dtype=np.float32)
    maskf = np.asarray(mask).astype(np.float32)
    out = np.empty((B, N, S, COUT), np.float32)
    for k in range(NCORES):
        s = s_list[k]
        den = s[:, :NCOL].reshape(2, CIN, NBLK, NQ)
        num = s[:, NCOL:].reshape(2, CIN, NBLK, NQ)
        # (h, c, blk, q) -> (blk, s1 = 2q + h, c)
        den = den.transpose(2, 3, 0, 1).reshape(NBLK, S, CIN)
        num = num.transpose(2, 3, 0, 1).reshape(NBLK, S, CIN)
        sl = slice(ISHARD * k, ISHARD * (k + 1))
        y_q = y[:, sl].reshape(NBLK, S, CIN)
        m_q = maskf[:, sl].reshape(NBLK, S)
        res = (y_q + num / den) * m_q[..., None]
        res = res @ w_lin.T
        out[:, sl] = res.reshape(B, ISHARD, S, COUT)
    return out


def kernel(pairwise_g, coset_functions, mask, w_y, b_y, w_g, b_g, w_lin):
    pairwise_g = np.asarray(pairwise_g, dtype=np.float32)
    coset_functions = np.asarray(coset_functions, dtype=np.float32)
    mask = np.asarray(mask)
    w_y = np.asarray(w_y, dtype=np.float32)
    w_g = np.asarray(w_g, dtype=np.float32)
    w_lin = np.asarray(w_lin, dtype=np.float32)

    nc = _get_program()
    in_maps = _host_prep(pairwise_g, coset_functions, mask, w_y, w_g)
    res = run_bass_kernel_spmd(nc, in_maps, core_ids=list(range(NCORES)))
    s_list = [r["out_s"] for r in res.results]
    return _host_finish(s_list, coset_functions, mask, w_lin)
